# revision 2
# baseline (speedup 1.0000x reference)
"""GPT-2-small (B=2,T=1024,E=768,L=12,H=12,V=50304) forward on 8 trn2 NeuronCores.

Sharding: DP=2 over batch (cores 0-3 = batch0, 4-7 = batch1); within a group,
sequence-parallel over tokens: core (g, r) owns canonical 128-token chunks
(r, 7-r) of its batch. Row-wise ops are token-local with full weights streamed
from HBM; attention gathers K/V within the group via two AllGathers per layer.
lm_head is vocab-parallel (each core computes its batch x 12576 vocab columns).

v2 vs baseline: no rank-1 bias matmuls (biases ride evictions / residual
pre-adds via partition-broadcast DMA), 2 xbar transposes per LN instead of 12,
single-DMA gather rearranges in local slot order, batched w2/lm-out DMAs,
K-proj scheduled first so both AllGathers overlap V/Q projections, DMA issue
split across both HWDGE rings (SP + ACT).
"""

import numpy as np
import ml_dtypes

import concourse.bacc as bacc
import concourse.bass as bass
import concourse.tile as tile
import concourse.mybir as mybir
from concourse.bass import ds, ts
from concourse.bass_utils import run_bass_kernel_spmd

F32 = mybir.dt.float32
BF16 = mybir.dt.bfloat16
AF = mybir.ActivationFunctionType
OP = mybir.AluOpType

B, T, V, E, L, H = 2, 1024, 50304, 768, 12, 12
HS = 64
P = 128
KO = 6            # E / 128
FCK = 24          # 3072 / 128
VS = V // 4       # 12576 vocab shard
VPAD = 12800      # padded to 25*512
NLM = 25          # lm chunks of 512
RG = [[0, 1, 2, 3], [4, 5, 6, 7]]
EPS = 1e-5

_cache = {}
MARKS = []


def _bcast_rows(ap, nrows):
    """Partition-broadcast AP: read the same DRAM row(s) into nrows partitions."""
    return bass.AP(tensor=ap.tensor, offset=ap.offset,
                   ap=[[0, nrows]] + [list(q) for q in ap.ap][1:])


def _build(inner=1):
    import os as _os
    _NOAG = bool(int(_os.environ.get("KBENCH_NOAG", "0")))
    nc = bacc.Bacc("TRN2", target_bir_lowering=False, debug=False, num_devices=8)
    MARKS.clear()

    def mark(name):
        MARKS.append((name, nc.get_next_instruction_name()))

    # ---------------- DRAM I/O ----------------
    idx_d = nc.dram_tensor("idx", [256], mybir.dt.int32, kind="ExternalInput").ap()
    temb_d = nc.dram_tensor("temb", [V, E], BF16, kind="ExternalInput").ap()
    pos_d = nc.dram_tensor("pos", [256, E], BF16, kind="ExternalInput").ap()
    mask_d = nc.dram_tensor("masks", [12, P, P], BF16, kind="ExternalInput").ap()
    wkv_d = nc.dram_tensor("wkv", [L, P, 2, KO, E], BF16, kind="ExternalInput").ap()
    wqp_d = nc.dram_tensor("wqp", [L, P, 2, KO, E], BF16, kind="ExternalInput").ap()
    wfc_d = nc.dram_tensor("wfc", [L, 4, P, KO, E], BF16, kind="ExternalInput").ap()
    w2_d = nc.dram_tensor("w2", [L, FCK, P, E], BF16, kind="ExternalInput").ap()
    bqkfc_d = nc.dram_tensor("bqkfc", [L, P, 36], F32, kind="ExternalInput").ap()
    brow_d = nc.dram_tensor("brow", [L, 1, 3 * E], BF16, kind="ExternalInput").ap()
    lmw_d = nc.dram_tensor("lmw", [P, KO, VPAD], BF16, kind="ExternalInput").ap()
    lmb_d = nc.dram_tensor("lmb", [1, VPAD], F32, kind="ExternalInput").ap()
    out_d = nc.dram_tensor("logits", [1024, VS], F32, kind="ExternalOutput").ap()

    with tile.TileContext(nc) as tc:
        from contextlib import ExitStack
        gctx = ExitStack()
        # ---------------- pools ----------------
        singles = gctx.enter_context(tc.tile_pool(name="singles", bufs=1))
        pstat = gctx.enter_context(tc.tile_pool(name="pstat", bufs=4))
        pact = gctx.enter_context(tc.tile_pool(name="pact", bufs=2))
        pbias = gctx.enter_context(tc.tile_pool(name="pbias", bufs=2))
        dram = gctx.enter_context(tc.tile_pool(name="dram", bufs=1, space="DRAM"))

        # ---------------- constants / setup ----------------
        eps_sb = singles.tile([P, 1], F32, name="eps_sb")
        nc.vector.memset(eps_sb[:], EPS)
        mask_sb = singles.tile([P, 12, P], BF16, name="mask_sb")
        nc.sync.dma_start(mask_sb[:], mask_d.rearrange("s k q -> k s q"))
        idx_sb = singles.tile([P, 2], mybir.dt.int32, name="idx_sb")
        nc.sync.dma_start(idx_sb[:], idx_d.rearrange("(c p) -> p c", p=P))
        pos_sb = singles.tile([P, 2, E], BF16, name="pos_sb")
        nc.sync.dma_start(pos_sb[:], pos_d.rearrange("(c p) m -> p c m", p=P))

        # residual stream x: [128 tok, 2 chunks, 768] fp32, persistent
        x = singles.tile([P, 2, E], F32, name="x_res")

        def layernorm(xin, xout):
            """xin fp32 [128,2,768] -> xout bf16 [128,2,768] (pure (x-m)*rstd)."""
            for c in range(2):
                st = pstat.tile([P, 2, 6], F32, tag="st")
                xv = xin[:, c, :].rearrange("p (a b) -> p a b", b=384)
                for sg in range(2):
                    nc.vector.bn_stats(st[:, sg, :], xv[:, sg, :])
                mv = pstat.tile([P, 2], F32, tag="mv")
                nc.vector.bn_aggr(mv[:], st[:])
                rstd = pstat.tile([P, 1], F32, tag="rs")
                nc.scalar.activation(rstd[:], mv[:, 1:2], AF.Sqrt, bias=eps_sb[:], scale=1.0)
                nc.vector.reciprocal(rstd[:], rstd[:])
                nmr = pstat.tile([P, 1], F32, tag="nm")
                nc.vector.tensor_tensor(nmr[:], mv[:, 0:1], rstd[:], OP.mult)
                nc.vector.tensor_scalar_mul(nmr[:], nmr[:], -1.0)
                nc.scalar.activation(xout[:, c, :], xin[:, c, :], AF.Identity,
                                     bias=nmr[:], scale=rstd[:])

        def transpose_act(xh, tag):
            """bf16 [128,2,768] token-major -> [128,6,256] feature-major."""
            xhT = pact.tile([P, KO, 256], BF16, tag=tag)
            for c in range(2):
                nc.scalar.dma_start_transpose(
                    xhT[:, :, ts(c, P)], xh[:, c, :])
            return xhT

        lctx = ExitStack()
        pwkv = lctx.enter_context(tc.tile_pool(name="pwkv", bufs=2))
        pwqp = lctx.enter_context(tc.tile_pool(name="pwqp", bufs=1))
        pwfc = lctx.enter_context(tc.tile_pool(name="pwfc", bufs=2))
        pw2 = lctx.enter_context(tc.tile_pool(name="pw2", bufs=2))
        pqkv = lctx.enter_context(tc.tile_pool(name="pqkv", bufs=1))
        pkv = lctx.enter_context(tc.tile_pool(name="pkv", bufs=1))
        patt = lctx.enter_context(tc.tile_pool(name="patt", bufs=6))
        phT = lctx.enter_context(tc.tile_pool(name="phT", bufs=1))
        ps_s = lctx.enter_context(tc.tile_pool(name="ps_s", bufs=4, space="PSUM"))
        ps_o = lctx.enter_context(tc.tile_pool(name="ps_o", bufs=2, space="PSUM"))
        ps_big = lctx.enter_context(tc.tile_pool(name="ps_big", bufs=2, space="PSUM"))
        emb_sb = singles.tile([P, 2, E], BF16, name="emb_sb")

        def embed():
            for c in range(2):
                nc.gpsimd.indirect_dma_start(
                    out=emb_sb[:, c, :], out_offset=None,
                    in_=temb_d,
                    in_offset=bass.IndirectOffsetOnAxis(ap=idx_sb[:, c:c + 1], axis=0),
                )
                nc.vector.tensor_tensor(x[:, c, :], emb_sb[:, c, :],
                                        pos_sb[:, c, :], OP.add)

        # ---------------- transformer layers ----------------
        for li in range(inner * L):
            l = li % L
            if l == 0:
                embed()
            mark(f"L{li}.start")
            wkv_sb = pwkv.tile([P, 2, KO, E], BF16, tag="wkv")
            nc.sync.dma_start(wkv_sb[:], wkv_d[l])
            wqp_sb = pwqp.tile([P, 2, KO, E], BF16, tag="wqp")
            nc.sync.dma_start(wqp_sb[:], wqp_d[l])
            bqkfc_sb = pbias.tile([P, 36], F32, tag="bqkfc")
            nc.sync.dma_start(bqkfc_sb[:], bqkfc_d[l])
            bbc_sb = pbias.tile([P, 3, E], BF16, tag="bbc")
            nc.scalar.dma_start(bbc_sb[:].rearrange("p a e -> p (a e)"),
                                _bcast_rows(brow_d[l], P))

            # LN1 + transpose
            xh = pact.tile([P, 2, E], BF16, tag="xh")
            layernorm(x, xh)
            xhT = transpose_act(xh, "xhT")
            mark(f"L{li}.kproj")

            # K projection (feature-major) + V projection into one KV buffer,
            # then a single merged AllGather per layer
            KVW = KO * 256 + 2 * H * (HS + 1)
            kv_loc = pqkv.tile([P, KVW], BF16, tag="kv")
            for m in range(KO):
                pm = ps_big.tile([P, 512], F32, tag="big")
                for kk in range(KO):
                    nc.tensor.matmul(pm[:, :256], wkv_sb[:, 0, kk, ts(m, P)],
                                     xhT[:, kk, :], start=(kk == 0), stop=(kk == 5))
                nc.scalar.activation(kv_loc[:, ts(m, 256)], pm[:, :256], AF.Identity,
                                     bias=bqkfc_sb[:, m:m + 1], scale=1.0)
            mark(f"L{li}.vproj")
            vplus = kv_loc[:, KO * 256:].rearrange("p (c h d) -> p c h d", c=2, h=H)
            nc.vector.memset(vplus[:, :, :, HS:HS + 1], 1.0)
            for tt in range(2):
                for c0, cw in ((0, 512), (512, 256)):
                    pm = ps_big.tile([P, 512], F32, tag="big")
                    for kk in range(KO):
                        nc.tensor.matmul(pm[:, :cw], xhT[:, kk, ts(tt, P)],
                                         wkv_sb[:, 1, kk, c0:c0 + cw],
                                         start=(kk == 0), stop=(kk == 5))
                    nc.vector.tensor_tensor(
                        vplus[:, tt, c0 // HS:(c0 + cw) // HS, 0:HS],
                        pm[:, :cw].rearrange("p (h d) -> p h d", d=HS),
                        bbc_sb[:, 0, c0:c0 + cw].rearrange("p (h d) -> p h d", d=HS),
                        OP.add)
            mark(f"L{li}.agkv")
            agkv_i = dram.tile([P, KVW], BF16, name=f"agkvi{li}")
            agkv_o = dram.tile([4, P, KVW], BF16, name=f"agkvo{li}")
            nc.sync.dma_start(agkv_i[:], kv_loc[:])
            if _NOAG:
                for _rr in range(4):
                    nc.sync.dma_start(agkv_o[_rr], agkv_i[:])
            else:
                nc.gpsimd.collective_compute(
                    "AllGather", OP.bypass, replica_groups=RG,
                    ins=[agkv_i[:].opt()], outs=[agkv_o[:].opt()])
            # rearranges into local slot order
            kT_all = pkv.tile([P, 4, KO, 2, P], BF16, tag="kTa")
            nc.scalar.dma_start(
                kT_all[:], agkv_o[:, :, 0:KO * 256]
                .rearrange("r p (a c e) -> p r a c e", a=KO, c=2))
            vplus_all = pkv.tile([P, 4, 2, H, HS + 1], BF16, tag="vpa")
            nc.scalar.dma_start(
                vplus_all[:],
                agkv_o[:, :, KO * 256:].rearrange("r p (c h d) -> p r c h d",
                                                  c=2, h=H))

            mark(f"L{li}.qproj")
            # Q projection (feature-major output)
            qT = pqkv.tile([P, KO, 256], BF16, tag="qT")
            for m in range(KO):
                pm = ps_big.tile([P, 512], F32, tag="big")
                for kk in range(KO):
                    nc.tensor.matmul(pm[:, :256], wqp_sb[:, 0, kk, ts(m, P)],
                                     xhT[:, kk, :], start=(kk == 0), stop=(kk == 5))
                nc.scalar.activation(qT[:, m, :], pm[:, :256], AF.Identity,
                                     bias=bqkfc_sb[:, 6 + m:7 + m], scale=1.0)

            # residual pre-add of proj bias (off critical path)
            for tt in range(2):
                nc.vector.tensor_tensor(x[:, tt, :], x[:, tt, :],
                                        bbc_sb[:, 1, :], OP.add)

            mark(f"L{li}.att")
            # ---------------- attention ----------------
            # gs 0..7: paired slots on even local key slots (rr=gs//2, cc=0),
            #          even gs -> q chunk 0 cols, odd gs -> q chunk 1 cols.
            # gs 8..11: B-only slots on odd local key slots (rr=gs-8, cc=1).
            attT = patt.tile([P, KO, 256], BF16, tag="attT", bufs=1)
            for sub in range(KO):
                hE, hO = 2 * sub, 2 * sub + 1
                pts = {hE: [], hO: []}
                for grp in range(3):
                    pss = {}
                    for h, base in ((hE, 0), (hO, HS)):
                        pss[h] = ps_s.tile([P, 512], F32, tag="s", name=f"pss{h}g")
                    for s4 in range(4):
                        gs = grp * 4 + s4
                        rr, cc = (gs // 2, 0) if gs < 8 else (gs - 8, 1)
                        qr = (0, P) if (gs < 8 and gs % 2 == 0) else (P, 256)
                        for h, base in ((hE, 0), (hO, HS)):
                            nc.tensor.matmul(pss[h][:, ts(s4, P)],
                                             kT_all[base:base + HS, rr, sub, cc, :],
                                             qT[base:base + HS, sub, qr[0]:qr[1]],
                                             start=True, stop=True)
                    for h in (hE, hO):
                        pt = patt.tile([P, 4, P], BF16, tag="pt", bufs=12)
                        nc.scalar.activation(
                            pt[:], pss[h][:].rearrange("p (a b) -> p a b", a=4),
                            AF.Exp, scale=HS ** -0.5)
                        nc.vector.tensor_tensor(
                            pt[:], pt[:], mask_sb[:, 4 * grp:4 * grp + 4, :], OP.mult)
                        pts[h].append(pt)
                psos = {hE: ps_o.tile([HS + 1, 256], F32, tag="o", name="psoE"),
                        hO: ps_o.tile([HS + 1, 256], F32, tag="o", name="psoO")}
                for j in range(8):
                    for h in (hE, hO):
                        if j % 2 == 0:
                            m = j // 2
                            rhs = pts[h][m // 2][:, (m % 2) * 2:(m % 2) * 2 + 2, :]
                            nc.tensor.matmul(psos[h][:, 0:256],
                                             vplus_all[:, j // 2, 0, h, :],
                                             rhs, start=(j == 0), stop=False)
                        else:
                            rhs = pts[h][2][:, (j - 1) // 2, :]
                            nc.tensor.matmul(psos[h][:, P:256],
                                             vplus_all[:, (j - 1) // 2, 1, h, :],
                                             rhs, start=False, stop=(j == 7))
                # softmax normalization: batched reciprocal-broadcast per sub
                rc = patt.tile([1, 2, 256], F32, tag="rc", bufs=2)
                for hi, h in enumerate((hE, hO)):
                    nc.vector.reciprocal(rc[:, hi, :], psos[h][HS:HS + 1, :])
                rcd = dram.tile([1, 2, 256], F32, tag="rcd", bufs=2)
                nc.sync.dma_start(rcd[:], rc[:])
                rcb = patt.tile([HS, 2, 256], F32, tag="rcb", bufs=2)
                nc.scalar.dma_start(rcb[:], _bcast_rows(rcd[:], HS))
                nc.vector.tensor_tensor(attT[0:HS, sub, :], psos[hE][0:HS, :],
                                        rcb[:, 0, :], OP.mult)
                ot = patt.tile([HS, 256], BF16, tag="ot", bufs=2)
                nc.vector.tensor_tensor(ot[:], psos[hO][0:HS, :], rcb[:, 1, :],
                                        OP.mult)
                nc.scalar.dma_start(attT[HS:P, sub, :], ot[:])

            mark(f"L{li}.proj")
            # output projection + residual (bias was pre-added to x)
            for tt in range(2):
                for c0, cw in ((0, 512), (512, 256)):
                    pm = ps_big.tile([P, 512], F32, tag="big")
                    for kk in range(KO):
                        nc.tensor.matmul(pm[:, :cw], attT[:, kk, ts(tt, P)],
                                         wqp_sb[:, 1, kk, c0:c0 + cw],
                                         start=(kk == 0), stop=(kk == 5))
                    nc.vector.tensor_tensor(x[:, tt, c0:c0 + cw], x[:, tt, c0:c0 + cw],
                                            pm[:, :cw], OP.add)

            mark(f"L{li}.ffn")
            # ---------------- FFN ----------------
            xh2 = pact.tile([P, 2, E], BF16, tag="xh")
            layernorm(x, xh2)
            xh2T = transpose_act(xh2, "xhT")
            # residual pre-add of FFN-down bias
            for tt in range(2):
                nc.vector.tensor_tensor(x[:, tt, :], x[:, tt, :],
                                        bbc_sb[:, 2, :], OP.add)
            hT = phT.tile([P, FCK, 256], BF16, tag="hT")
            for ci in range(4):
                wfc_sb = pwfc.tile([P, KO, E], BF16, tag="w")
                nc.sync.dma_start(wfc_sb[:], wfc_d[l, ci])
                for mm in range(KO):
                    ch = ci * KO + mm
                    pm = ps_big.tile([P, 512], F32, tag="big")
                    for kk in range(KO):
                        nc.tensor.matmul(pm[:, :256], wfc_sb[:, kk, ts(mm, P)],
                                         xh2T[:, kk, :], start=(kk == 0), stop=(kk == 5))
                    nc.scalar.activation(hT[:, ch, :], pm[:, :256], AF.Relu,
                                         bias=bqkfc_sb[:, 12 + ch:13 + ch], scale=1.0)
            mark(f"L{li}.w2")
            pms = {}
            for tt in range(2):
                for ci, (c0, cw) in enumerate(((0, 512), (512, 256))):
                    pms[(tt, ci)] = ps_s.tile([P, 512], F32, tag="s", name=f"w2pm{tt}{ci}")
            for kb in range(6):
                w2_sb = pw2.tile([P, 4, E], BF16, tag="w2")
                nc.sync.dma_start(w2_sb[:], w2_d[l, 4 * kb:4 * kb + 4]
                                  .rearrange("k p e -> p k e"))
                for kl in range(4):
                    kk = 4 * kb + kl
                    for tt in range(2):
                        for ci, (c0, cw) in enumerate(((0, 512), (512, 256))):
                            nc.tensor.matmul(pms[(tt, ci)][:, :cw],
                                             hT[:, kk, ts(tt, P)],
                                             w2_sb[:, kl, c0:c0 + cw],
                                             start=(kk == 0), stop=(kk == FCK - 1))
            for tt in range(2):
                for ci, (c0, cw) in enumerate(((0, 512), (512, 256))):
                    nc.vector.tensor_tensor(x[:, tt, c0:c0 + cw], x[:, tt, c0:c0 + cw],
                                            pms[(tt, ci)][:, :cw], OP.add)

        # ---------------- final LN + AllGather + lm_head ----------------
        mark("final")
        xfTs = []
        for frep in range(inner):
            xhf = pact.tile([P, 2, E], BF16, tag="xh")
            layernorm(x, xhf)
            xhfT = transpose_act(xhf, "xhT")
            agf_i = dram.tile([P, KO * 256], BF16, name=f"agfi{frep}")
            agf_o = dram.tile([4, P, KO * 256], BF16, name=f"agfo{frep}")
            nc.sync.dma_start(agf_i[:].rearrange("p (a b) -> p a b", a=KO), xhfT[:])
            if _NOAG:
                for _rr in range(4):
                    nc.sync.dma_start(agf_o[_rr], agf_i[:])
            else:
                nc.gpsimd.collective_compute(
                    "AllGather", OP.bypass, replica_groups=RG,
                    ins=[agf_i[:].opt()], outs=[agf_o[:].opt()])
            xfT = pkv.tile([P, 4, KO, 2, P], BF16, tag="kTa")
            nc.scalar.dma_start(
                xfT[:], agf_o[:].rearrange("r p (a c e) -> p r a c e", a=KO, c=2))
            xfTs.append(xfT)

        lctx.close()
        mark("lm")

        with tc.tile_pool(name="plm", bufs=3) as plm, \
             tc.tile_pool(name="plog", bufs=4) as plog, \
             tc.tile_pool(name="ps_lm", bufs=6, space="PSUM") as ps_lm:
          for frep in range(inner):
            xfT = xfTs[frep]
            for chk in range(NLM):
                lw = plm.tile([P, KO, 512], BF16, tag="lw")
                N = 512 if chk < NLM - 1 else VS - 512 * (NLM - 1)
                nc.sync.dma_start(lw[:], lmw_d[:, :, ts(chk, 512)])
                lmbbc = plm.tile([P, 512], F32, tag="lmbbc")
                nc.scalar.dma_start(lmbbc[:, :N],
                                    _bcast_rows(lmb_d[:, 512 * chk:512 * chk + N], P))
                for tg in range(2):
                    lg = plog.tile([P, 4, 512], F32, tag="lg")
                    for t4 in range(4):
                        tt = 4 * tg + t4
                        # xfT is in local slot order; map canonical chunk tt
                        # to its local slot s
                        s = 2 * tt if tt < 4 else 15 - 2 * tt
                        pm = ps_lm.tile([P, 512], F32, tag="lm")
                        for kk in range(KO):
                            nc.tensor.matmul(pm[:, :N],
                                             xfT[:, s // 2, kk, s % 2, :],
                                             lw[:, kk, :N], start=(kk == 0),
                                             stop=(kk == 5))
                        nc.vector.tensor_tensor(lg[:, t4, :N], pm[:, :N],
                                                lmbbc[:, :N], OP.add)
                    nc.sync.dma_start(
                        out_d[512 * tg:512 * (tg + 1), 512 * chk:512 * chk + N]
                        .rearrange("(g p) n -> p g n", p=P),
                        lg[:, :, :N])
        gctx.close()

    nc.compile()
    return nc


def _prep(inputs):
    bf = ml_dtypes.bfloat16
    f = np.float32
    g = lambda k: np.asarray(inputs[k], f)
    idx = np.asarray(inputs["idx"]).astype(np.int32)
    wq, wk, wv, wproj = g("wq"), g("wk"), g("wv"), g("wproj")
    g1, b1, g2, b2 = g("ln1_g"), g("ln1_b"), g("ln2_g"), g("ln2_b")
    wfc, wpr2 = g("wfc"), g("wpr2")
    bfc, bproj, bpr2 = g("bfc"), g("bproj"), g("bpr2")
    gf, bff = g("lnf_g"), g("lnf_b")
    lm_w, lm_b = g("lm_w"), g("lm_b")

    wq_e = g1[:, :, None] * wq
    wk_e = g1[:, :, None] * wk
    wv_e = g1[:, :, None] * wv
    wfc_e = g2[:, :, None] * wfc
    bq_e = np.einsum("le,leo->lo", b1, wq)
    bk_e = np.einsum("le,leo->lo", b1, wk)
    bv_e = np.einsum("le,leo->lo", b1, wv)
    bfc_e = bfc + np.einsum("le,leo->lo", b2, wfc)
    lmw_e = gf[:, None] * lm_w
    lmb_e = lm_b + bff @ lm_w

    def pack(w):  # [L,768,N] -> [L,128,6,N]
        Lx, Ex, Nx = w.shape
        return np.ascontiguousarray(
            w.reshape(Lx, KO, P, Nx).transpose(0, 2, 1, 3)).astype(bf)

    bqkfc = np.concatenate(
        [bk_e.reshape(L, KO, P).transpose(0, 2, 1),
         bq_e.reshape(L, KO, P).transpose(0, 2, 1),
         bfc_e.reshape(L, FCK, P).transpose(0, 2, 1)], axis=2).astype(f)

    com = {
        "temb": np.asarray(inputs["tok_emb"], f).astype(bf),
        "wkv": np.ascontiguousarray(
            np.stack([pack(wk_e), pack(wv_e)], axis=2)),
        "wqp": np.ascontiguousarray(
            np.stack([pack(wq_e), pack(wproj)], axis=2)),
        "wfc": np.ascontiguousarray(
            pack(wfc_e).reshape(L, P, KO, 4, E).transpose(0, 3, 1, 2, 4)),
        "w2": wpr2.reshape(L, FCK, P, E).astype(bf),
        "bqkfc": bqkfc,
        "brow": np.concatenate([bv_e, bproj, bpr2], axis=1)[:, None, :].astype(bf),
    }
    pos = np.asarray(inputs["pos_emb"], f).astype(bf)

    in_maps = []
    ar = np.arange(P)
    for core in range(8):
        gb, r = divmod(core, 4)
        c1, c2 = r, 7 - r
        m = dict(com)
        sl = lmw_e[:, r * VS:(r + 1) * VS]
        lmw_pad = np.zeros((E, VPAD), f)
        lmw_pad[:, :VS] = sl
        m["lmw"] = np.ascontiguousarray(
            lmw_pad.reshape(KO, P, VPAD).transpose(1, 0, 2)).astype(bf)
        lmb_pad = np.zeros((1, VPAD), f)
        lmb_pad[0, :VS] = lmb_e[r * VS:(r + 1) * VS]
        m["lmb"] = lmb_pad
        m["idx"] = np.concatenate(
            [idx[gb, c1 * P:(c1 + 1) * P], idx[gb, c2 * P:(c2 + 1) * P]])
        m["pos"] = np.concatenate(
            [pos[c1 * P:(c1 + 1) * P], pos[c2 * P:(c2 + 1) * P]])
        # slots 0..7: canonical chunks 0..3 x (q chunk c1, c2);
        # slots 8..11: canonical chunks 7,6,5,4 vs q chunk c2 (local odd slots)
        masks = np.zeros((12, P, P), f)
        for j in range(4):
            masks[2 * j] = (j * P + ar[:, None]) <= (c1 * P + ar[None, :])
            masks[2 * j + 1] = (j * P + ar[:, None]) <= (c2 * P + ar[None, :])
        for i in range(4):
            masks[8 + i] = ((7 - i) * P + ar[:, None]) <= (c2 * P + ar[None, :])
        m["masks"] = masks.astype(bf)
        in_maps.append(m)
    return in_maps


LAST_RESULTS = None
LAST_TIME_NS = None


def _timed_run(nc, in_maps, reps=3):
    """Replicates bass2jax.run_bass_via_pjrt's multi-core path, but keeps
    inputs device-resident so repeated executions time the NEFF itself."""
    import time as _time
    import jax
    from jax.experimental.shard_map import shard_map
    from jax.sharding import Mesh, PartitionSpec, NamedSharding
    from concourse import bass2jax as b2j
    import concourse.mybir as _mb

    b2j.install_neuronx_cc_hook()
    n_cores = len(in_maps)
    partition_name = nc.partition_id_tensor.name if nc.partition_id_tensor else None
    in_names, out_names, out_avals, zero_outs = [], [], [], []
    for alloc in nc.m.functions[0].allocations:
        if not isinstance(alloc, _mb.MemoryLocationSet):
            continue
        name = alloc.memorylocations[0].name
        if alloc.kind == "ExternalInput":
            if name != partition_name:
                in_names.append(name)
        elif alloc.kind == "ExternalOutput":
            out_names.append(name)
            shape = tuple(alloc.tensor_shape)
            dtype = _mb.dt.np(alloc.dtype)
            out_avals.append(jax.core.ShapedArray(shape, dtype))
            zero_outs.append(np.zeros(shape, dtype))
    n_params = len(in_names)
    n_outs = len(out_avals)
    in_names.extend(out_names)
    if partition_name is not None:
        in_names.append(partition_name)
    donate = tuple(range(n_params, n_params + n_outs))

    def _body(*args):
        operands = list(args)
        if partition_name is not None:
            operands.append(b2j.partition_id_tensor())
        return tuple(b2j._bass_exec_p.bind(
            *operands, out_avals=tuple(out_avals), in_names=tuple(in_names),
            out_names=tuple(out_names), lowering_input_output_aliases=(),
            sim_require_finite=True, sim_require_nnan=True, nc=nc))

    devices = jax.devices()[:n_cores]
    mesh = Mesh(np.asarray(devices), ("core",))
    spec = PartitionSpec("core")
    sharded = jax.jit(
        shard_map(_body, mesh=mesh, in_specs=(spec,) * (n_params + n_outs),
                  out_specs=(spec,) * n_outs, check_rep=False),
        donate_argnums=donate, keep_unused=True)
    sh = NamedSharding(mesh, spec)
    concat_in = [
        jax.device_put(
            np.concatenate([np.asarray(in_maps[c][nm]) for c in range(n_cores)], axis=0),
            sh)
        for nm in in_names[:n_params]]
    jax.block_until_ready(concat_in)
    times = []
    out_arrs = None
    for rep in range(reps):
        zeros_dev = [
            jax.device_put(np.zeros((n_cores * z.shape[0], *z.shape[1:]), z.dtype), sh)
            for z in zero_outs]
        jax.block_until_ready(zeros_dev)
        t0 = _time.perf_counter()
        out_arrs = sharded(*concat_in, *zeros_dev)
        jax.block_until_ready(out_arrs)
        times.append(_time.perf_counter() - t0)
    results = [
        {nm: np.asarray(out_arrs[i]).reshape(n_cores, *out_avals[i].shape)[c]
         for i, nm in enumerate(out_names)}
        for c in range(n_cores)]
    return results, times


def kernel(**inputs):
    global LAST_RESULTS, LAST_TIME_NS
    import os
    inner = int(os.environ.get("KBENCH_INNER", "1"))
    if ("nc", inner) not in _cache:
        _cache[("nc", inner)] = _build(inner)
    nc = _cache[("nc", inner)]
    in_maps = _prep(inputs)
    reps = int(os.environ.get("KBENCH_TIME_REPS", "0"))
    if reps > 0:
        results, times = _timed_run(nc, in_maps, reps=reps)
        LAST_TIME_NS = int(min(times) * 1e9)
        LAST_RESULTS = None
    else:
        res = run_bass_kernel_spmd(nc, in_maps, core_ids=list(range(8)))
        LAST_RESULTS = res
        results = res.results
    out = np.zeros((B, T, V), np.float32)
    for core in range(8):
        gb, r = divmod(core, 4)
        out[gb, :, r * VS:(r + 1) * VS] = results[core]["logits"]
    return out


# revision 3
# speedup vs baseline: 1.0787x; 1.0787x over previous
"""GPT-2-small (B=2,T=1024,E=768,L=12,H=12,V=50304) forward on 8 trn2 NeuronCores.

Sharding: DP=2 over batch (cores 0-3 = batch0, 4-7 = batch1); within a group,
sequence-parallel over tokens: core (g, r) owns canonical 128-token chunks
(r, 7-r) of its batch. Row-wise ops are token-local with full weights streamed
from HBM; attention gathers K/V within the group via two AllGathers per layer.
lm_head is vocab-parallel (each core computes its batch x 12576 vocab columns).

v2 vs baseline: no rank-1 bias matmuls (biases ride evictions / residual
pre-adds via partition-broadcast DMA), 2 xbar transposes per LN instead of 12,
single-DMA gather rearranges in local slot order, batched w2/lm-out DMAs,
K-proj scheduled first so both AllGathers overlap V/Q projections, DMA issue
split across both HWDGE rings (SP + ACT).
"""

import numpy as np
import ml_dtypes

import concourse.bacc as bacc
import concourse.bass as bass
import concourse.tile as tile
import concourse.mybir as mybir
from concourse.bass import ds, ts
from concourse.bass_utils import run_bass_kernel_spmd

F32 = mybir.dt.float32
BF16 = mybir.dt.bfloat16
AF = mybir.ActivationFunctionType
OP = mybir.AluOpType

B, T, V, E, L, H = 2, 1024, 50304, 768, 12, 12
HS = 64
P = 128
KO = 6            # E / 128
FCK = 24          # 3072 / 128
VS = V // 4       # 12576 vocab shard
VPAD = 12800      # padded to 25*512
NLM = 25          # lm chunks of 512
RG = [[0, 1, 2, 3], [4, 5, 6, 7]]
EPS = 1e-5

_cache = {}
MARKS = []


def _bcast_rows(ap, nrows):
    """Partition-broadcast AP: read the same DRAM row(s) into nrows partitions."""
    return bass.AP(tensor=ap.tensor, offset=ap.offset,
                   ap=[[0, nrows]] + [list(q) for q in ap.ap][1:])


def _build(inner=1):
    import os as _os
    _NOAG = bool(int(_os.environ.get("KBENCH_NOAG", "0")))
    nc = bacc.Bacc("TRN2", target_bir_lowering=False, debug=False, num_devices=8)
    MARKS.clear()

    def mark(name):
        MARKS.append((name, nc.get_next_instruction_name()))

    # ---------------- DRAM I/O ----------------
    idx_d = nc.dram_tensor("idx", [256], mybir.dt.int32, kind="ExternalInput").ap()
    temb_d = nc.dram_tensor("temb", [V, E], BF16, kind="ExternalInput").ap()
    pos_d = nc.dram_tensor("pos", [256, E], BF16, kind="ExternalInput").ap()
    mask_d = nc.dram_tensor("masks", [12, P, P], BF16, kind="ExternalInput").ap()
    wkv_d = nc.dram_tensor("wkv", [L, P, 2, KO, E], BF16, kind="ExternalInput").ap()
    wqp_d = nc.dram_tensor("wqp", [L, P, 2, KO, E], BF16, kind="ExternalInput").ap()
    wfc_d = nc.dram_tensor("wfc", [L, 4, P, KO, E], BF16, kind="ExternalInput").ap()
    w2_d = nc.dram_tensor("w2", [L, FCK, P, E], BF16, kind="ExternalInput").ap()
    bqkfc_d = nc.dram_tensor("bqkfc", [L, P, 36], F32, kind="ExternalInput").ap()
    brow_d = nc.dram_tensor("brow", [L, 1, 3 * E], BF16, kind="ExternalInput").ap()
    lmw_d = nc.dram_tensor("lmw", [P, KO, VPAD], BF16, kind="ExternalInput").ap()
    lmb_d = nc.dram_tensor("lmb", [1, VPAD], F32, kind="ExternalInput").ap()
    out_d = nc.dram_tensor("logits", [1024, VS], F32, kind="ExternalOutput").ap()

    with tile.TileContext(nc) as tc:
        from contextlib import ExitStack
        gctx = ExitStack()
        # ---------------- pools ----------------
        singles = gctx.enter_context(tc.tile_pool(name="singles", bufs=1))
        pstat = gctx.enter_context(tc.tile_pool(name="pstat", bufs=4))
        pact = gctx.enter_context(tc.tile_pool(name="pact", bufs=2))
        pbias = gctx.enter_context(tc.tile_pool(name="pbias", bufs=2))
        dram = gctx.enter_context(tc.tile_pool(name="dram", bufs=1, space="DRAM"))

        # ---------------- constants / setup ----------------
        eps_sb = singles.tile([P, 1], F32, name="eps_sb")
        nc.vector.memset(eps_sb[:], EPS)
        mask_sb = singles.tile([P, 12, P], BF16, name="mask_sb")
        nc.sync.dma_start(mask_sb[:], mask_d.rearrange("s k q -> k s q"))
        idx_sb = singles.tile([P, 2], mybir.dt.int32, name="idx_sb")
        nc.sync.dma_start(idx_sb[:], idx_d.rearrange("(c p) -> p c", p=P))
        pos_sb = singles.tile([P, 2, E], BF16, name="pos_sb")
        nc.sync.dma_start(pos_sb[:], pos_d.rearrange("(c p) m -> p c m", p=P))

        # residual stream x: [128 tok, 2 chunks, 768] fp32, persistent
        x = singles.tile([P, 2, E], F32, name="x_res")

        def layernorm(xin, xout):
            """xin fp32 [128,2,768] -> xout bf16 [128,2,768] (pure (x-m)*rstd)."""
            for c in range(2):
                st = pstat.tile([P, 2, 6], F32, tag="st")
                xv = xin[:, c, :].rearrange("p (a b) -> p a b", b=384)
                for sg in range(2):
                    nc.vector.bn_stats(st[:, sg, :], xv[:, sg, :])
                mv = pstat.tile([P, 2], F32, tag="mv")
                nc.vector.bn_aggr(mv[:], st[:])
                rstd = pstat.tile([P, 1], F32, tag="rs")
                nc.scalar.activation(rstd[:], mv[:, 1:2], AF.Sqrt, bias=eps_sb[:], scale=1.0)
                nc.vector.reciprocal(rstd[:], rstd[:])
                nmr = pstat.tile([P, 1], F32, tag="nm")
                nc.vector.tensor_tensor(nmr[:], mv[:, 0:1], rstd[:], OP.mult)
                nc.vector.tensor_scalar_mul(nmr[:], nmr[:], -1.0)
                nc.scalar.activation(xout[:, c, :], xin[:, c, :], AF.Identity,
                                     bias=nmr[:], scale=rstd[:])

        def transpose_act(xh, tag):
            """bf16 [128,2,768] token-major -> [128,6,256] feature-major."""
            xhT = pact.tile([P, KO, 256], BF16, tag=tag)
            for c in range(2):
                nc.scalar.dma_start_transpose(
                    xhT[:, :, ts(c, P)], xh[:, c, :])
            return xhT

        lctx = ExitStack()
        pwkv = lctx.enter_context(tc.tile_pool(name="pwkv", bufs=2))
        pwqp = lctx.enter_context(tc.tile_pool(name="pwqp", bufs=1))
        pwfc = lctx.enter_context(tc.tile_pool(name="pwfc", bufs=2))
        pw2 = lctx.enter_context(tc.tile_pool(name="pw2", bufs=2))
        pqkv = lctx.enter_context(tc.tile_pool(name="pqkv", bufs=1))
        pkv = lctx.enter_context(tc.tile_pool(name="pkv", bufs=1))
        patt = lctx.enter_context(tc.tile_pool(name="patt", bufs=6))
        phT = lctx.enter_context(tc.tile_pool(name="phT", bufs=1))
        ps_s = lctx.enter_context(tc.tile_pool(name="ps_s", bufs=4, space="PSUM"))
        ps_o = lctx.enter_context(tc.tile_pool(name="ps_o", bufs=2, space="PSUM"))
        ps_big = lctx.enter_context(tc.tile_pool(name="ps_big", bufs=2, space="PSUM"))
        emb_sb = singles.tile([P, 2, E], BF16, name="emb_sb")

        def embed():
            for c in range(2):
                nc.gpsimd.indirect_dma_start(
                    out=emb_sb[:, c, :], out_offset=None,
                    in_=temb_d,
                    in_offset=bass.IndirectOffsetOnAxis(ap=idx_sb[:, c:c + 1], axis=0),
                )
                nc.vector.tensor_tensor(x[:, c, :], emb_sb[:, c, :],
                                        pos_sb[:, c, :], OP.add)

        # ---------------- transformer layers ----------------
        for li in range(inner * L):
            l = li % L
            if l == 0:
                embed()
            mark(f"L{li}.start")
            wkv_sb = pwkv.tile([P, 2, KO, E], BF16, tag="wkv")
            nc.sync.dma_start(wkv_sb[:], wkv_d[l])
            wqp_sb = pwqp.tile([P, 2, KO, E], BF16, tag="wqp")
            nc.sync.dma_start(wqp_sb[:], wqp_d[l])
            bqkfc_sb = pbias.tile([P, 36], F32, tag="bqkfc")
            nc.sync.dma_start(bqkfc_sb[:], bqkfc_d[l])
            bbc_sb = pbias.tile([P, 3, E], BF16, tag="bbc")
            nc.scalar.dma_start(bbc_sb[:].rearrange("p a e -> p (a e)"),
                                _bcast_rows(brow_d[l], P))

            # LN1 + transpose
            xh = pact.tile([P, 2, E], BF16, tag="xh")
            layernorm(x, xh)
            xhT = transpose_act(xh, "xhT")
            mark(f"L{li}.kproj")

            # K projection (feature-major) + V projection into one KV buffer,
            # then a single merged AllGather per layer
            KVW = KO * 256 + 2 * H * (HS + 1)
            kv_loc = pqkv.tile([P, KVW], BF16, tag="kv")
            for m in range(KO):
                pm = ps_big.tile([P, 512], F32, tag="big")
                for kk in range(KO):
                    nc.tensor.matmul(pm[:, :256], wkv_sb[:, 0, kk, ts(m, P)],
                                     xhT[:, kk, :], start=(kk == 0), stop=(kk == 5))
                nc.scalar.activation(kv_loc[:, ts(m, 256)], pm[:, :256], AF.Identity,
                                     bias=bqkfc_sb[:, m:m + 1], scale=1.0)
            mark(f"L{li}.vproj")
            vplus = kv_loc[:, KO * 256:].rearrange("p (c h d) -> p c h d", c=2, h=H)
            nc.vector.memset(vplus[:, :, :, HS:HS + 1], 1.0)
            for tt in range(2):
                for c0, cw in ((0, 512), (512, 256)):
                    pm = ps_big.tile([P, 512], F32, tag="big")
                    for kk in range(KO):
                        nc.tensor.matmul(pm[:, :cw], xhT[:, kk, ts(tt, P)],
                                         wkv_sb[:, 1, kk, c0:c0 + cw],
                                         start=(kk == 0), stop=(kk == 5))
                    nc.vector.tensor_tensor(
                        vplus[:, tt, c0 // HS:(c0 + cw) // HS, 0:HS],
                        pm[:, :cw].rearrange("p (h d) -> p h d", d=HS),
                        bbc_sb[:, 0, c0:c0 + cw].rearrange("p (h d) -> p h d", d=HS),
                        OP.add)
            mark(f"L{li}.agkv")
            agkv_i = dram.tile([P, KVW], BF16, name=f"agkvi{li}")
            agkv_o = dram.tile([4, P, KVW], BF16, name=f"agkvo{li}")
            nc.sync.dma_start(agkv_i[:], kv_loc[:])
            if _NOAG:
                for _rr in range(4):
                    nc.sync.dma_start(agkv_o[_rr], agkv_i[:])
            else:
                nc.gpsimd.collective_compute(
                    "AllGather", OP.bypass, replica_groups=RG,
                    ins=[agkv_i[:].opt()], outs=[agkv_o[:].opt()])
            # rearranges into local slot order
            kT_all = pkv.tile([P, 4, KO, 2, P], BF16, tag="kTa")
            nc.scalar.dma_start(
                kT_all[:], agkv_o[:, :, 0:KO * 256]
                .rearrange("r p (a c e) -> p r a c e", a=KO, c=2))
            vplus_all = pkv.tile([P, 4, 2, H, HS + 1], BF16, tag="vpa")
            nc.scalar.dma_start(
                vplus_all[:],
                agkv_o[:, :, KO * 256:].rearrange("r p (c h d) -> p r c h d",
                                                  c=2, h=H))

            mark(f"L{li}.qproj")
            # Q projection (feature-major output)
            qT = pqkv.tile([P, KO, 256], BF16, tag="qT")
            for m in range(KO):
                pm = ps_big.tile([P, 512], F32, tag="big")
                for kk in range(KO):
                    nc.tensor.matmul(pm[:, :256], wqp_sb[:, 0, kk, ts(m, P)],
                                     xhT[:, kk, :], start=(kk == 0), stop=(kk == 5))
                nc.scalar.activation(qT[:, m, :], pm[:, :256], AF.Identity,
                                     bias=bqkfc_sb[:, 6 + m:7 + m], scale=1.0)

            # residual pre-add of proj bias (off critical path)
            for tt in range(2):
                nc.vector.tensor_tensor(x[:, tt, :], x[:, tt, :],
                                        bbc_sb[:, 1, :], OP.add)

            mark(f"L{li}.att")
            # ---------------- attention ----------------
            # gs 0..7: paired slots on even local key slots (rr=gs//2, cc=0),
            #          even gs -> q chunk 0 cols, odd gs -> q chunk 1 cols.
            # gs 8..11: B-only slots on odd local key slots (rr=gs-8, cc=1).
            attT = patt.tile([P, KO, 256], BF16, tag="attT", bufs=1)
            ot_all = patt.tile([HS, KO, 256], BF16, tag="ot", bufs=1)
            for sub in range(KO):
                hE, hO = 2 * sub, 2 * sub + 1
                pts = {hE: [], hO: []}
                for grp in range(3):
                    pss = {}
                    for h, base in ((hE, 0), (hO, HS)):
                        pss[h] = ps_s.tile([P, 512], F32, tag="s", name=f"pss{h}g")
                    if grp < 2:
                        # paired A/B slots share the key stationary and write
                        # adjacent 256 PSUM columns -> one N=256 matmul each
                        for pp in range(2):
                            rr = 2 * grp + pp
                            for h, base in ((hE, 0), (hO, HS)):
                                nc.tensor.matmul(pss[h][:, ts(pp, 256)],
                                                 kT_all[base:base + HS, rr, sub, 0, :],
                                                 qT[base:base + HS, sub, :],
                                                 start=True, stop=True)
                    else:
                        for s4 in range(4):
                            gs = 8 + s4
                            for h, base in ((hE, 0), (hO, HS)):
                                nc.tensor.matmul(pss[h][:, ts(s4, P)],
                                                 kT_all[base:base + HS, gs - 8, sub, 1, :],
                                                 qT[base:base + HS, sub, P:256],
                                                 start=True, stop=True)
                    for h in (hE, hO):
                        pt = patt.tile([P, 4, P], BF16, tag="pt", bufs=12)
                        nc.scalar.activation(
                            pt[:], pss[h][:].rearrange("p (a b) -> p a b", a=4),
                            AF.Exp, scale=HS ** -0.5)
                        nc.vector.tensor_tensor(
                            pt[:], pt[:], mask_sb[:, 4 * grp:4 * grp + 4, :], OP.mult)
                        pts[h].append(pt)
                psos = {hE: ps_o.tile([HS + 1, 256], F32, tag="o", name="psoE"),
                        hO: ps_o.tile([HS + 1, 256], F32, tag="o", name="psoO")}
                for j in range(8):
                    for h in (hE, hO):
                        if j % 2 == 0:
                            m = j // 2
                            rhs = pts[h][m // 2][:, (m % 2) * 2:(m % 2) * 2 + 2, :]
                            nc.tensor.matmul(psos[h][:, 0:256],
                                             vplus_all[:, j // 2, 0, h, :],
                                             rhs, start=(j == 0), stop=False)
                        else:
                            rhs = pts[h][2][:, (j - 1) // 2, :]
                            nc.tensor.matmul(psos[h][:, P:256],
                                             vplus_all[:, (j - 1) // 2, 1, h, :],
                                             rhs, start=False, stop=(j == 7))
                # softmax normalization: batched reciprocal-broadcast per sub
                rc = patt.tile([1, 2, 256], F32, tag="rc", bufs=2)
                for hi, h in enumerate((hE, hO)):
                    nc.vector.reciprocal(rc[:, hi, :], psos[h][HS:HS + 1, :])
                rcd = dram.tile([1, 2, 256], F32, tag="rcd", bufs=2)
                nc.sync.dma_start(rcd[:], rc[:])
                rcb = patt.tile([HS, 2, 256], F32, tag="rcb", bufs=2)
                nc.scalar.dma_start(rcb[:], _bcast_rows(rcd[:], HS))
                nc.vector.tensor_tensor(attT[0:HS, sub, :], psos[hE][0:HS, :],
                                        rcb[:, 0, :], OP.mult)
                nc.vector.tensor_tensor(ot_all[:, sub, :], psos[hO][0:HS, :],
                                        rcb[:, 1, :], OP.mult)
            nc.scalar.dma_start(attT[HS:P, :, :], ot_all[:])

            mark(f"L{li}.proj")
            # output projection + residual (bias was pre-added to x)
            for tt in range(2):
                for c0, cw in ((0, 512), (512, 256)):
                    pm = ps_big.tile([P, 512], F32, tag="big")
                    for kk in range(KO):
                        nc.tensor.matmul(pm[:, :cw], attT[:, kk, ts(tt, P)],
                                         wqp_sb[:, 1, kk, c0:c0 + cw],
                                         start=(kk == 0), stop=(kk == 5))
                    nc.vector.tensor_tensor(x[:, tt, c0:c0 + cw], x[:, tt, c0:c0 + cw],
                                            pm[:, :cw], OP.add)

            mark(f"L{li}.ffn")
            # ---------------- FFN ----------------
            xh2 = pact.tile([P, 2, E], BF16, tag="xh")
            layernorm(x, xh2)
            xh2T = transpose_act(xh2, "xhT")
            # residual pre-add of FFN-down bias
            for tt in range(2):
                nc.vector.tensor_tensor(x[:, tt, :], x[:, tt, :],
                                        bbc_sb[:, 2, :], OP.add)
            hT = phT.tile([P, FCK, 256], BF16, tag="hT")
            for ci in range(4):
                wfc_sb = pwfc.tile([P, KO, E], BF16, tag="w")
                nc.sync.dma_start(wfc_sb[:], wfc_d[l, ci])
                for mm in range(KO):
                    ch = ci * KO + mm
                    pm = ps_big.tile([P, 512], F32, tag="big")
                    for kk in range(KO):
                        nc.tensor.matmul(pm[:, :256], wfc_sb[:, kk, ts(mm, P)],
                                         xh2T[:, kk, :], start=(kk == 0), stop=(kk == 5))
                    nc.scalar.activation(hT[:, ch, :], pm[:, :256], AF.Relu,
                                         bias=bqkfc_sb[:, 12 + ch:13 + ch], scale=1.0)
            mark(f"L{li}.w2")
            pms = {}
            for tt in range(2):
                for ci, (c0, cw) in enumerate(((0, 512), (512, 256))):
                    pms[(tt, ci)] = ps_s.tile([P, 512], F32, tag="s", name=f"w2pm{tt}{ci}")
            for kb in range(6):
                w2_sb = pw2.tile([P, 4, E], BF16, tag="w2")
                nc.sync.dma_start(w2_sb[:], w2_d[l, 4 * kb:4 * kb + 4]
                                  .rearrange("k p e -> p k e"))
                for kl in range(4):
                    kk = 4 * kb + kl
                    for tt in range(2):
                        for ci, (c0, cw) in enumerate(((0, 512), (512, 256))):
                            nc.tensor.matmul(pms[(tt, ci)][:, :cw],
                                             hT[:, kk, ts(tt, P)],
                                             w2_sb[:, kl, c0:c0 + cw],
                                             start=(kk == 0), stop=(kk == FCK - 1))
            for tt in range(2):
                for ci, (c0, cw) in enumerate(((0, 512), (512, 256))):
                    nc.vector.tensor_tensor(x[:, tt, c0:c0 + cw], x[:, tt, c0:c0 + cw],
                                            pms[(tt, ci)][:, :cw], OP.add)

        # ---------------- final LN + AllGather + lm_head ----------------
        mark("final")
        xfTs = []
        for frep in range(inner):
            xhf = pact.tile([P, 2, E], BF16, tag="xh")
            layernorm(x, xhf)
            xhfT = transpose_act(xhf, "xhT")
            agf_i = dram.tile([P, KO * 256], BF16, name=f"agfi{frep}")
            agf_o = dram.tile([4, P, KO * 256], BF16, name=f"agfo{frep}")
            nc.sync.dma_start(agf_i[:].rearrange("p (a b) -> p a b", a=KO), xhfT[:])
            if _NOAG:
                for _rr in range(4):
                    nc.sync.dma_start(agf_o[_rr], agf_i[:])
            else:
                nc.gpsimd.collective_compute(
                    "AllGather", OP.bypass, replica_groups=RG,
                    ins=[agf_i[:].opt()], outs=[agf_o[:].opt()])
            xfT = pkv.tile([P, 4, KO, 2, P], BF16, tag="kTa")
            nc.scalar.dma_start(
                xfT[:], agf_o[:].rearrange("r p (a c e) -> p r a c e", a=KO, c=2))
            xfTs.append(xfT)

        lctx.close()
        mark("lm")

        with tc.tile_pool(name="plm", bufs=3) as plm, \
             tc.tile_pool(name="plog", bufs=4) as plog, \
             tc.tile_pool(name="ps_lm", bufs=6, space="PSUM") as ps_lm:
          for frep in range(inner):
            xfT = xfTs[frep]
            for chk in range(NLM):
                lw = plm.tile([P, KO, 512], BF16, tag="lw")
                N = 512 if chk < NLM - 1 else VS - 512 * (NLM - 1)
                nc.sync.dma_start(lw[:], lmw_d[:, :, ts(chk, 512)])
                lmbbc = plm.tile([P, 512], F32, tag="lmbbc")
                nc.scalar.dma_start(lmbbc[:, :N],
                                    _bcast_rows(lmb_d[:, 512 * chk:512 * chk + N], P))
                for tg in range(2):
                    lg = plog.tile([P, 4, 512], F32, tag="lg")
                    for t4 in range(4):
                        tt = 4 * tg + t4
                        # xfT is in local slot order; map canonical chunk tt
                        # to its local slot s
                        s = 2 * tt if tt < 4 else 15 - 2 * tt
                        pm = ps_lm.tile([P, 512], F32, tag="lm")
                        for kk in range(KO):
                            nc.tensor.matmul(pm[:, :N],
                                             xfT[:, s // 2, kk, s % 2, :],
                                             lw[:, kk, :N], start=(kk == 0),
                                             stop=(kk == 5))
                        nc.vector.tensor_tensor(lg[:, t4, :N], pm[:, :N],
                                                lmbbc[:, :N], OP.add)
                    nc.sync.dma_start(
                        out_d[512 * tg:512 * (tg + 1), 512 * chk:512 * chk + N]
                        .rearrange("(g p) n -> p g n", p=P),
                        lg[:, :, :N])
        gctx.close()

    nc.compile()
    return nc


def _prep(inputs):
    bf = ml_dtypes.bfloat16
    f = np.float32
    g = lambda k: np.asarray(inputs[k], f)
    idx = np.asarray(inputs["idx"]).astype(np.int32)
    wq, wk, wv, wproj = g("wq"), g("wk"), g("wv"), g("wproj")
    g1, b1, g2, b2 = g("ln1_g"), g("ln1_b"), g("ln2_g"), g("ln2_b")
    wfc, wpr2 = g("wfc"), g("wpr2")
    bfc, bproj, bpr2 = g("bfc"), g("bproj"), g("bpr2")
    gf, bff = g("lnf_g"), g("lnf_b")
    lm_w, lm_b = g("lm_w"), g("lm_b")

    wq_e = g1[:, :, None] * wq
    wk_e = g1[:, :, None] * wk
    wv_e = g1[:, :, None] * wv
    wfc_e = g2[:, :, None] * wfc
    bq_e = np.einsum("le,leo->lo", b1, wq)
    bk_e = np.einsum("le,leo->lo", b1, wk)
    bv_e = np.einsum("le,leo->lo", b1, wv)
    bfc_e = bfc + np.einsum("le,leo->lo", b2, wfc)
    lmw_e = gf[:, None] * lm_w
    lmb_e = lm_b + bff @ lm_w

    def pack(w):  # [L,768,N] -> [L,128,6,N]
        Lx, Ex, Nx = w.shape
        return np.ascontiguousarray(
            w.reshape(Lx, KO, P, Nx).transpose(0, 2, 1, 3)).astype(bf)

    bqkfc = np.concatenate(
        [bk_e.reshape(L, KO, P).transpose(0, 2, 1),
         bq_e.reshape(L, KO, P).transpose(0, 2, 1),
         bfc_e.reshape(L, FCK, P).transpose(0, 2, 1)], axis=2).astype(f)

    com = {
        "temb": np.asarray(inputs["tok_emb"], f).astype(bf),
        "wkv": np.ascontiguousarray(
            np.stack([pack(wk_e), pack(wv_e)], axis=2)),
        "wqp": np.ascontiguousarray(
            np.stack([pack(wq_e), pack(wproj)], axis=2)),
        "wfc": np.ascontiguousarray(
            pack(wfc_e).reshape(L, P, KO, 4, E).transpose(0, 3, 1, 2, 4)),
        "w2": wpr2.reshape(L, FCK, P, E).astype(bf),
        "bqkfc": bqkfc,
        "brow": np.concatenate([bv_e, bproj, bpr2], axis=1)[:, None, :].astype(bf),
    }
    pos = np.asarray(inputs["pos_emb"], f).astype(bf)

    in_maps = []
    ar = np.arange(P)
    for core in range(8):
        gb, r = divmod(core, 4)
        c1, c2 = r, 7 - r
        m = dict(com)
        sl = lmw_e[:, r * VS:(r + 1) * VS]
        lmw_pad = np.zeros((E, VPAD), f)
        lmw_pad[:, :VS] = sl
        m["lmw"] = np.ascontiguousarray(
            lmw_pad.reshape(KO, P, VPAD).transpose(1, 0, 2)).astype(bf)
        lmb_pad = np.zeros((1, VPAD), f)
        lmb_pad[0, :VS] = lmb_e[r * VS:(r + 1) * VS]
        m["lmb"] = lmb_pad
        m["idx"] = np.concatenate(
            [idx[gb, c1 * P:(c1 + 1) * P], idx[gb, c2 * P:(c2 + 1) * P]])
        m["pos"] = np.concatenate(
            [pos[c1 * P:(c1 + 1) * P], pos[c2 * P:(c2 + 1) * P]])
        # slots 0..7: canonical chunks 0..3 x (q chunk c1, c2);
        # slots 8..11: canonical chunks 7,6,5,4 vs q chunk c2 (local odd slots)
        masks = np.zeros((12, P, P), f)
        for j in range(4):
            masks[2 * j] = (j * P + ar[:, None]) <= (c1 * P + ar[None, :])
            masks[2 * j + 1] = (j * P + ar[:, None]) <= (c2 * P + ar[None, :])
        for i in range(4):
            masks[8 + i] = ((7 - i) * P + ar[:, None]) <= (c2 * P + ar[None, :])
        m["masks"] = masks.astype(bf)
        in_maps.append(m)
    return in_maps


LAST_RESULTS = None
LAST_TIME_NS = None


def _timed_run(nc, in_maps, reps=3):
    """Replicates bass2jax.run_bass_via_pjrt's multi-core path, but keeps
    inputs device-resident so repeated executions time the NEFF itself."""
    import time as _time
    import jax
    from jax.experimental.shard_map import shard_map
    from jax.sharding import Mesh, PartitionSpec, NamedSharding
    from concourse import bass2jax as b2j
    import concourse.mybir as _mb

    b2j.install_neuronx_cc_hook()
    n_cores = len(in_maps)
    partition_name = nc.partition_id_tensor.name if nc.partition_id_tensor else None
    in_names, out_names, out_avals, zero_outs = [], [], [], []
    for alloc in nc.m.functions[0].allocations:
        if not isinstance(alloc, _mb.MemoryLocationSet):
            continue
        name = alloc.memorylocations[0].name
        if alloc.kind == "ExternalInput":
            if name != partition_name:
                in_names.append(name)
        elif alloc.kind == "ExternalOutput":
            out_names.append(name)
            shape = tuple(alloc.tensor_shape)
            dtype = _mb.dt.np(alloc.dtype)
            out_avals.append(jax.core.ShapedArray(shape, dtype))
            zero_outs.append(np.zeros(shape, dtype))
    n_params = len(in_names)
    n_outs = len(out_avals)
    in_names.extend(out_names)
    if partition_name is not None:
        in_names.append(partition_name)
    donate = tuple(range(n_params, n_params + n_outs))

    def _body(*args):
        operands = list(args)
        if partition_name is not None:
            operands.append(b2j.partition_id_tensor())
        return tuple(b2j._bass_exec_p.bind(
            *operands, out_avals=tuple(out_avals), in_names=tuple(in_names),
            out_names=tuple(out_names), lowering_input_output_aliases=(),
            sim_require_finite=True, sim_require_nnan=True, nc=nc))

    devices = jax.devices()[:n_cores]
    mesh = Mesh(np.asarray(devices), ("core",))
    spec = PartitionSpec("core")
    sharded = jax.jit(
        shard_map(_body, mesh=mesh, in_specs=(spec,) * (n_params + n_outs),
                  out_specs=(spec,) * n_outs, check_rep=False),
        donate_argnums=donate, keep_unused=True)
    sh = NamedSharding(mesh, spec)
    concat_in = [
        jax.device_put(
            np.concatenate([np.asarray(in_maps[c][nm]) for c in range(n_cores)], axis=0),
            sh)
        for nm in in_names[:n_params]]
    jax.block_until_ready(concat_in)
    times = []
    out_arrs = None
    for rep in range(reps):
        zeros_dev = [
            jax.device_put(np.zeros((n_cores * z.shape[0], *z.shape[1:]), z.dtype), sh)
            for z in zero_outs]
        jax.block_until_ready(zeros_dev)
        t0 = _time.perf_counter()
        out_arrs = sharded(*concat_in, *zeros_dev)
        jax.block_until_ready(out_arrs)
        times.append(_time.perf_counter() - t0)
    results = [
        {nm: np.asarray(out_arrs[i]).reshape(n_cores, *out_avals[i].shape)[c]
         for i, nm in enumerate(out_names)}
        for c in range(n_cores)]
    return results, times


def kernel(**inputs):
    global LAST_RESULTS, LAST_TIME_NS
    import os
    inner = int(os.environ.get("KBENCH_INNER", "1"))
    if ("nc", inner) not in _cache:
        _cache[("nc", inner)] = _build(inner)
    nc = _cache[("nc", inner)]
    in_maps = _prep(inputs)
    reps = int(os.environ.get("KBENCH_TIME_REPS", "0"))
    if reps > 0:
        results, times = _timed_run(nc, in_maps, reps=reps)
        LAST_TIME_NS = int(min(times) * 1e9)
        LAST_RESULTS = None
    else:
        res = run_bass_kernel_spmd(nc, in_maps, core_ids=list(range(8)))
        LAST_RESULTS = res
        results = res.results
    out = np.zeros((B, T, V), np.float32)
    for core in range(8):
        gb, r = divmod(core, 4)
        out[gb, :, r * VS:(r + 1) * VS] = results[core]["logits"]
    return out
